# revision 7
# baseline (speedup 1.0000x reference)
"""DisentangledSelfAttention (DeBERTa-style) Trainium2 Bass kernel.

Sharding: 8 cores = 4 batch-pairs x 2 head-halves.  Core c handles batches
(2*(c%4), 2*(c%4)+1) and heads [6*(c//4), 6*(c//4)+6).  Each core emits the
partial output sum over its 6 heads for its 2 batches; the host adds the two
head-half partials per batch (standard tensor-parallel gather).  This halves
the replicated positional projections (Kp/Qp) versus pure batch parallelism.

Key algebraic structure exploited (same as the v1 kernel):
  rel[i, j] = j - i + 511 depends only on (j - i); for S=384 only rel rows
  128..894 (767 values) are used.  qp[i,p] = q[i].Kp[p] is bounced to DRAM
  [384x512] per (batch,head) and read back with row pitch 511 ("skew" read)
  which turns the per-row diagonal shift into a flat strided access; same for
  kq[j,p'] = k[j].Qp_rev[p'], read back transposed and accumulated into the
  score PSUM via identity matmuls.

Cost-model-driven choices:
  - All matmul operands are bf16 (1 cycle/row incl. transposes and short-N;
    halves every DMA transfer and enables DVE 2x/4x modes).  PSUM stays f32.
  - softmax scale, q_bias are folded into Wq/bq on host; v_bias+bv are folded
    through Wo into the output bias on host (softmax rows sum to 1).
  - c2p is added into the score PSUM by an identity matmul (PE is cheaper
    than a DVE tensor_tensor on a 4-byte PSUM operand).
  - weight/activation uploads are single rearranged DMAs (HWDGE acquisition
    costs ~625ns per DMA instruction).
  - PSUM->SBUF copies are spread across DVE / Act / Pool engines.
"""

import os
import sys

import numpy as np

B, S, D, H = 8, 384, 768, 12
DH = D // H          # 64
MAX_POS = 512
NP = 767             # used relative positions (128..894)
SCALE = DH ** -0.5

NB = 2               # batches per core
NH = 6               # heads per core
DHALF = NH * DH      # 384 projection columns per core
NIT = S // 128       # 3 i/j tiles per batch
NKT = D // 128       # 6 contraction tiles over D
NOT = DHALF // 128   # 3 output tiles over the head half
NPP = 768            # positional axis padded to even
NW = 512             # per-i-tile window of the positional axis (511 used)
S2 = NB * S          # 768 tokens per core (2 batches)

_CACHE = {}


def _import_concourse():
    try:
        import concourse.bass  # noqa: F401
    except ImportError:
        for p in ("/opt/trn_rl_repo", "/root/.axon_site/_ro/trn_rl_repo"):
            if os.path.isdir(p) and p not in sys.path:
                sys.path.insert(0, p)
        import concourse.bass  # noqa: F401


def _build():
    """Build + finalize the per-core Bass program (identical on all cores)."""
    _import_concourse()
    import concourse.bass as bass
    import concourse.bacc as bacc
    import concourse.mybir as mybir
    import concourse.tile as tile
    from concourse.bass import ts
    from concourse.masks import make_identity
    from concourse.tile import add_dep_helper

    f32 = mybir.dt.float32
    bf16 = mybir.dt.bfloat16
    ADD = mybir.AluOpType.add
    EXP = mybir.ActivationFunctionType.Exp

    nc = bacc.Bacc("TRN2", target_bir_lowering=False, debug=False)

    # ---------------- DRAM I/O ----------------
    xT = nc.dram_tensor("xT", [D, S2], bf16, kind="ExternalInput")
    wq = nc.dram_tensor("wq", [D, DHALF], bf16, kind="ExternalInput")
    wk = nc.dram_tensor("wk", [D, DHALF], bf16, kind="ExternalInput")
    wv = nc.dram_tensor("wv", [D, DHALF], bf16, kind="ExternalInput")
    wpk = nc.dram_tensor("wpk", [D, DHALF], bf16, kind="ExternalInput")
    wpq = nc.dram_tensor("wpq", [D, DHALF], bf16, kind="ExternalInput")
    wo = nc.dram_tensor("wo", [DHALF, D], bf16, kind="ExternalInput")
    relkT = nc.dram_tensor("relkT", [D, NPP], bf16, kind="ExternalInput")
    bq = nc.dram_tensor("bq", [DHALF], f32, kind="ExternalInput")
    bk = nc.dram_tensor("bk", [DHALF], f32, kind="ExternalInput")
    bo = nc.dram_tensor("bo", [D], f32, kind="ExternalInput")
    out = nc.dram_tensor("out", [S2, D], f32, kind="ExternalOutput")

    NI = NB * NH     # 12 (batch, head) instances
    qp_dram = [nc.dram_tensor(f"qp_scratch_{i}", [S, NW], bf16) for i in range(NI)]
    kq_dram = [nc.dram_tensor(f"kq_scratch_{i}", [S, NW], bf16) for i in range(NI)]

    with tile.TileContext(nc) as tc:
        with (
            tc.tile_pool(name="const", bufs=1) as constp,
            tc.tile_pool(name="big", bufs=1) as bigp,
            tc.tile_pool(name="wpool", bufs=2) as wpool,
            tc.tile_pool(name="psA", bufs=3, space="PSUM") as psA,
            tc.tile_pool(name="psSC", bufs=2, space="PSUM") as psSC,
            tc.tile_pool(name="psWT", bufs=2, space="PSUM") as psWT,
            tc.tile_pool(name="psAV", bufs=1, space="PSUM") as psAV,
        ):
            def psum(tag, shape=None, dtype=f32):
                pool = {"ps": psA, "sc": psSC, "wtps": psWT, "avps": psAV}[tag]
                return pool.tile(shape or [128, NW], dtype, tag=tag, name=tag)

            qT_sb = bigp.tile([128, NOT, S2], bf16, tag="qT")
            kT_sb = bigp.tile([128, NOT, S2], bf16, tag="kT")
            v_sb = bigp.tile([128, NB * NIT, DHALF], bf16, tag="v")
            KpT_sb = bigp.tile([128, NOT, NPP], bf16, tag="KpT")
            QpTr_sb = bigp.tile([128, NOT, NPP], bf16, tag="QpTr")
            attnT_sb = bigp.tile([128, NOT, S2], bf16, tag="attnT")

            # ---------- stage 1+2: projections & positional projections ----
            with tc.tile_pool(name="bigtmp", bufs=2) as bigtmp:
                xT_sb = bigtmp.tile([128, NKT, S2], bf16, tag="bigtmp")
                nc.sync.dma_start(
                    xT_sb[:], xT[:].rearrange("(o p) c -> p o c", p=128)
                )
                ident = constp.tile([128, 128], bf16, tag="ident")
                make_identity(nc, ident[:])

                bq_sb = constp.tile([128, NOT], f32, tag="bq")
                bk_sb = constp.tile([128, NOT], f32, tag="bk")
                bof = constp.tile([1, D], f32, tag="bo")
                bor = constp.tile([128, D], f32, tag="bor")
                nc.sync.dma_start(bq_sb[:], bq[:].rearrange("(o p) -> p o", p=128))
                nc.sync.dma_start(bk_sb[:], bk[:].rearrange("(o p) -> p o", p=128))
                nc.sync.dma_start(bof[:], bo[:].unsqueeze(0))
                nc.gpsimd.partition_broadcast(bor[:], bof[:])

                # q^T and k^T : [dout(part), i]  (bias per-partition)
                for wdram, bias_sb, dst in ((wq, bq_sb, qT_sb), (wk, bk_sb, kT_sb)):
                    w_sb = wpool.tile([128, NKT, DHALF], bf16, tag="w")
                    nc.sync.dma_start(
                        w_sb[:], wdram[:].rearrange("(o p) c -> p o c", p=128)
                    )
                    for mo in range(NOT):
                        for b in range(NB):
                            ps_t = psum("ps")
                            for ko in range(NKT):
                                nc.tensor.matmul(
                                    ps_t[:, :S],
                                    w_sb[:, ko, ts(mo, 128)],
                                    xT_sb[:, ko, b * S : (b + 1) * S],
                                    start=(ko == 0),
                                    stop=(ko == NKT - 1),
                                )
                            nc.vector.tensor_scalar_add(
                                dst[:, mo, b * S : (b + 1) * S],
                                ps_t[:, :S],
                                bias_sb[:, mo : mo + 1],
                            )

                # v : [j(part), dh]  (no bias: v_bias+bv folded into bo on host)
                w_sb = wpool.tile([128, NKT, DHALF], bf16, tag="w")
                nc.sync.dma_start(
                    w_sb[:], wv[:].rearrange("(o p) c -> p o c", p=128)
                )
                for b in range(NB):
                    for jt in range(NIT):
                        ps_t = psum("ps")
                        for ko in range(NKT):
                            nc.tensor.matmul(
                                ps_t[:, :DHALF],
                                xT_sb[:, ko, b * S + 128 * jt : b * S + 128 * (jt + 1)],
                                w_sb[:, ko, :],
                                start=(ko == 0),
                                stop=(ko == NKT - 1),
                            )
                        nc.scalar.copy(
                            v_sb[:, NIT * b + jt, :], ps_t[:, :DHALF]
                        )

                # Kp^T and QpRev^T : [dout(part), p].  Reversed rel operand is
                # built on-chip from the forward copy (negative-step DVE copy).
                relk_keep = None
                for idx, (wdram, dst) in enumerate(((wpk, KpT_sb), (wpq, QpTr_sb))):
                    w_sb = wpool.tile([128, NKT, DHALF], bf16, tag="w")
                    nc.sync.dma_start(
                        w_sb[:], wdram[:].rearrange("(o p) c -> p o c", p=128)
                    )
                    rel_sb = bigtmp.tile(
                        [128, NKT, NPP], bf16, tag="bigtmp", name=f"rel{idx}"
                    )
                    if idx == 0:
                        nc.sync.dma_start(
                            rel_sb[:], relkT[:].rearrange("(o p) c -> p o c", p=128)
                        )
                        relk_keep = rel_sb
                    else:
                        # rel_rev[p'] = rel_fwd[766 - p']; col 767 copied from
                        # the forward tile's zero pad
                        nc.vector.tensor_copy(
                            rel_sb[:, :, NPP - 1 : NPP],
                            relk_keep[:, :, NPP - 1 : NPP],
                        )
                        for ko in range(NKT):
                            fwd = relk_keep[:, ko, 0 : NPP - 1]
                            rev = bass.AP(
                                fwd.tensor,
                                fwd.offset + (NPP - 2),
                                [[fwd.ap[0][0], 128], [-1, NPP - 1]],
                            )
                            nc.vector.tensor_copy(rel_sb[:, ko, 0 : NPP - 1], rev)
                    for mo in range(NOT):
                        for ci in range(2):
                            cs = 384 * ci
                            ps_t = psum("ps")
                            for ko in range(NKT):
                                nc.tensor.matmul(
                                    ps_t[:, :384],
                                    w_sb[:, ko, ts(mo, 128)],
                                    rel_sb[:, ko, cs : cs + 384],
                                    start=(ko == 0),
                                    stop=(ko == NKT - 1),
                                )
                            if (mo + ci) % 2 == 0:
                                nc.vector.tensor_copy(
                                    dst[:, mo, cs : cs + 384], ps_t[:, :384]
                                )
                            else:
                                nc.scalar.copy(
                                    dst[:, mo, cs : cs + 384], ps_t[:, :384]
                                )

                # prefetch Wo: fills the stage2->3 DMA lull
                wo_sb = wpool.tile([128, NOT, D], bf16, tag="w", name="wo_sb")
                nc.sync.dma_start(
                    wo_sb[:], wo[:].rearrange("(o p) c -> p o c", p=128)
                )

            # ---------- stages 3-5: attention per (batch, head) ------------
            with (
                tc.tile_pool(name="work", bufs=3) as workp,
                tc.tile_pool(name="small", bufs=4) as smallp,
            ):
                import concourse.bass as bass_mod

                qp_w = [None] * NI
                kq_w = [None] * NI
                # GPSIMD cannot access PSUM, so PSUM->SBUF copies alternate
                # between the DVE and Activation engines only.
                cp_engs = [nc.vector.tensor_copy, nc.scalar.copy]

                def inst_bh(i):
                    return i // NH, i % NH

                def stage3(i):
                    """qp/kq windowed matmuls + bounce to DRAM for instance i.

                    For i-tile t only positional columns [256-128t, 768-128t)
                    are ever read back, so each row tile computes a 512-wide
                    window; bounce rows are stored with pitch 512.
                    """
                    b, h = inst_bh(i)
                    hp, ho = 64 * (h % 2), h // 2
                    for which in range(2):  # 0 -> qp, 1 -> kq
                        sb = workp.tile(
                            [128, NIT, NW], bf16,
                            tag=f"bounce{which}", name=f"bounce{which}", bufs=3,
                        )
                        for it in range(NIT):
                            w0 = 256 - 128 * it
                            lhsT = (qT_sb if which == 0 else kT_sb)[
                                hp : hp + 64, ho, b * S + 128 * it : b * S + 128 * (it + 1)
                            ]
                            rhs = (KpT_sb if which == 0 else QpTr_sb)[
                                hp : hp + 64, ho, w0 : w0 + NW
                            ]
                            ps_t = psum("ps")
                            nc.tensor.matmul(
                                ps_t[:], lhsT, rhs, start=True, stop=True
                            )
                            cp_engs[(2 * i + which + it) % 2](
                                sb[:, it, :], ps_t[:]
                            )
                        dram = (qp_dram if which == 0 else kq_dram)[i]
                        w_inst = nc.sync.dma_start(
                            dram[:].rearrange("(o p) c -> p o c", p=128), sb[:]
                        )
                        if which == 0:
                            qp_w[i] = w_inst
                        else:
                            kq_w[i] = w_inst

                def stage45(i):
                    b, h = inst_bh(i)
                    hp, ho = 64 * (h % 2), h // 2
                    wT_sb = workp.tile([128, NIT, S], bf16, tag="wT")
                    # combined skew reads: c2p[t][ip, jf] and p2cT[t][u][jp, if]
                    # flat addr in [384, 512]: 127 + 511*row + 65536*tile + col
                    c2p_sb = workp.tile([128, NIT, S], bf16, tag="c2p", bufs=3)
                    r1 = nc.sync.dma_start(
                        c2p_sb[:],
                        bass_mod.AP(
                            qp_dram[i], 127,
                            [[511, 128], [128 * NW, NIT], [1, S]],
                        ),
                    )
                    add_dep_helper(r1.ins, qp_w[i].ins, reason="qp bounce")
                    # p2cT[u][jp, if] = kq[128u+jp, i-(128u+jp)+383]
                    p2ct_sb = workp.tile([128, NIT, S], bf16, tag="p2ct", bufs=3)
                    r2 = nc.sync.dma_start(
                        p2ct_sb[:],
                        bass_mod.AP(
                            kq_dram[i], 127,
                            [[511, 128], [128 * NW, NIT], [1, S]],
                        ),
                    )
                    add_dep_helper(r2.ins, kq_w[i].ins, reason="kq bounce")
                    for t in range(NIT):
                        # ---- scores psum: c2c + c2p (identity mm) + p2c^T
                        sc_ps = psum("sc", shape=[128, S])
                        nc.tensor.matmul(
                            sc_ps[:, :S],
                            qT_sb[hp : hp + 64, ho, b * S + 128 * t : b * S + 128 * (t + 1)],
                            kT_sb[hp : hp + 64, ho, b * S : (b + 1) * S],
                            start=True,
                            stop=False,
                            skip_group_check=True,
                        )
                        nc.tensor.matmul(
                            sc_ps[:, :S],
                            ident[:],
                            c2p_sb[:, t, :],
                            start=False,
                            stop=False,
                            skip_group_check=True,
                        )
                        for u in range(NIT):
                            # out[if, jf] = sum_jp p2ct[jp, if] * I[jp, jf]
                            nc.tensor.matmul(
                                sc_ps[:, ts(u, 128)],
                                p2ct_sb[:, u, ts(t, 128)],
                                ident[:],
                                start=False,
                                stop=(u == NIT - 1),
                                skip_group_check=True,
                            )
                        exp_sb = workp.tile([128, S], bf16, tag="exp", bufs=4)
                        ssum = smallp.tile([128, 1], f32, tag="ssum")
                        sinv = smallp.tile([128, 1], f32, tag="sinv")
                        nc.scalar.activation(
                            exp_sb[:], sc_ps[:, :S], EXP, accum_out=ssum[:]
                        )
                        nc.vector.reciprocal(sinv[:], ssum[:])
                        nc.vector.tensor_scalar_mul(exp_sb[:], exp_sb[:], sinv[:])
                        # ---- transpose normalized weights -> wT[j, i]
                        wt_ps = psum("wtps", shape=[128, S], dtype=bf16)
                        for u in range(NIT):
                            nc.tensor.matmul(
                                wt_ps[:, ts(u, 128)],
                                exp_sb[:, ts(u, 128)],
                                ident[:],
                                is_transpose=True,
                                skip_group_check=True,
                            )
                        cp_engs[(i + t) % 2](
                            wT_sb[:, :, ts(t, 128)],
                            wt_ps[:].rearrange("p (u c) -> p u c", u=NIT),
                        )
                    # ---- stage 5: AV for this instance -> attnT
                    av_ps = psum("avps", shape=[128, S])
                    for u in range(NIT):
                        nc.tensor.matmul(
                            av_ps[0:64, :S],
                            v_sb[:, NIT * b + u, 64 * h : 64 * (h + 1)],
                            wT_sb[:, u, :],
                            start=(u == 0),
                            stop=(u == NIT - 1),
                        )
                    cp_engs[i % 2](
                        attnT_sb[hp : hp + 64, ho, b * S : (b + 1) * S],
                        av_ps[0:64, :S],
                    )

                # software pipeline: keep PE fed while bounces land in DRAM
                stage3(0)
                stage3(1)
                for i in range(NI):
                    stage45(i)
                    if i + 2 < NI:
                        stage3(i + 2)

                # ---------- stage 6: output projection ---------------------
                for b in range(NB):
                    for it in range(NIT):
                        for no in range(2):
                            ps_t = psum("ps")
                            for ko in range(NOT):
                                nc.tensor.matmul(
                                    ps_t[:, :384],
                                    attnT_sb[:, ko, b * S + 128 * it : b * S + 128 * (it + 1)],
                                    wo_sb[:, ko, ts(no, 384)],
                                    start=(ko == 0),
                                    stop=(ko == NOT - 1),
                                )
                            o_sb = workp.tile([128, 384], f32, tag="osb")
                            nc.vector.tensor_tensor(
                                o_sb[:], ps_t[:, :384], bor[:, ts(no, 384)], ADD
                            )
                            nc.sync.dma_start(
                                out[b * S + 128 * it : b * S + 128 * (it + 1),
                                    ts(no, 384)],
                                o_sb[:],
                            )

    nc.finalize()
    return nc


def _get_program():
    if "nc" not in _CACHE:
        _CACHE["nc"] = _build()
    return _CACHE["nc"]


def _host_prep(inputs):
    import ml_dtypes

    f = np.float32
    bf = ml_dtypes.bfloat16
    x = np.asarray(inputs["x"], f)
    rel = np.asarray(inputs["rel_pos_emb"], f)
    rel_used = rel[MAX_POS - S : MAX_POS - S + NP]          # rows 128..894
    relkT = np.ascontiguousarray(
        np.pad(rel_used.T, ((0, 0), (0, 1)))).astype(bf)

    Wq = np.asarray(inputs["Wq"], f) * SCALE
    Wk = np.asarray(inputs["Wk"], f)
    Wv = np.asarray(inputs["Wv"], f)
    Wpk = np.asarray(inputs["Wpk"], f)
    Wpq = np.asarray(inputs["Wpq"], f) * SCALE
    Wo = np.asarray(inputs["Wo"], f)
    bq_full = (np.asarray(inputs["bq"], f) + np.asarray(inputs["q_bias"], f)) * SCALE
    bk_full = np.asarray(inputs["bk"], f)
    bvv = np.asarray(inputs["bv"], f) + np.asarray(inputs["v_bias"], f)
    bo_full = np.asarray(inputs["bo"], f)

    in_maps = []
    for c in range(B):
        bp, hh = c % 4, c // 4
        hs = slice(hh * DHALF, (hh + 1) * DHALF)
        b0, b1 = 2 * bp, 2 * bp + 1
        xT = np.concatenate([x[b0].T, x[b1].T], axis=1)
        # v_bias+bv pass through attention unchanged (softmax rows sum to 1)
        # and then through this core's half of Wo; bo itself added once (hh=0).
        bo_c = bvv[hs] @ Wo[hs] + (bo_full if hh == 0 else 0.0)
        in_maps.append({
            "xT": np.ascontiguousarray(xT).astype(bf),
            "wq": np.ascontiguousarray(Wq[:, hs]).astype(bf),
            "wk": np.ascontiguousarray(Wk[:, hs]).astype(bf),
            "wv": np.ascontiguousarray(Wv[:, hs]).astype(bf),
            "wpk": np.ascontiguousarray(Wpk[:, hs]).astype(bf),
            "wpq": np.ascontiguousarray(Wpq[:, hs]).astype(bf),
            "wo": np.ascontiguousarray(Wo[hs]).astype(bf),
            "relkT": relkT,
            "bq": np.ascontiguousarray(bq_full[hs]),
            "bk": np.ascontiguousarray(bk_full[hs]),
            "bo": bo_c.astype(f),
        })
    return in_maps


def _get_runner():
    """Build (once) a jitted SPMD executor for the compiled program."""
    key = "runner"
    if key in _CACHE:
        return _CACHE[key]
    _import_concourse()
    import jax
    from jax.sharding import Mesh, PartitionSpec
    from jax.experimental.shard_map import shard_map
    import concourse.mybir as mybir
    from concourse import bass2jax

    nc = _get_program()
    bass2jax.install_neuronx_cc_hook()

    partition_name = (
        nc.partition_id_tensor.name if nc.partition_id_tensor else None
    )
    in_names, out_names, out_avals, zero_outs = [], [], [], []
    for alloc in nc.m.functions[0].allocations:
        if not isinstance(alloc, mybir.MemoryLocationSet):
            continue
        name = alloc.memorylocations[0].name
        if alloc.kind == "ExternalInput":
            if name != partition_name:
                in_names.append(name)
        elif alloc.kind == "ExternalOutput":
            out_names.append(name)
            shape = tuple(alloc.tensor_shape)
            dtype = mybir.dt.np(alloc.dtype)
            out_avals.append(jax.core.ShapedArray(shape, dtype))
            zero_outs.append(np.zeros(shape, dtype))
    n_params = len(in_names)
    all_names = in_names + out_names
    if partition_name is not None:
        all_names = all_names + [partition_name]

    def _body(*args):
        operands = list(args)
        if partition_name is not None:
            operands.append(bass2jax.partition_id_tensor())
        outs = bass2jax._bass_exec_p.bind(
            *operands,
            out_avals=tuple(out_avals),
            in_names=tuple(all_names),
            out_names=tuple(out_names),
            lowering_input_output_aliases=(),
            sim_require_finite=True,
            sim_require_nnan=True,
            nc=nc,
        )
        return tuple(outs)

    devices = jax.devices()[:B]
    mesh = Mesh(np.asarray(devices), ("core",))
    n_outs = len(out_names)
    sharded = jax.jit(
        shard_map(
            _body,
            mesh=mesh,
            in_specs=(PartitionSpec("core"),) * (n_params + n_outs),
            out_specs=(PartitionSpec("core"),) * n_outs,
            check_rep=False,
        ),
        donate_argnums=tuple(range(n_params, n_params + n_outs)),
        keep_unused=True,
    )

    def run(in_maps):
        concat_in = [
            np.concatenate([np.asarray(in_maps[c][nm]) for c in range(B)], axis=0)
            for nm in in_names
        ]
        concat_zeros = [
            np.zeros((B * z.shape[0], *z.shape[1:]), z.dtype) for z in zero_outs
        ]
        out_arrs = sharded(*concat_in, *concat_zeros)
        return [
            {
                nm: np.asarray(out_arrs[i]).reshape(B, *out_avals[i].shape)[c]
                for i, nm in enumerate(out_names)
            }
            for c in range(B)
        ]

    _CACHE[key] = run
    return run


def _run(inputs, trace=False):
    run = _get_runner()
    in_maps = _host_prep(inputs)
    results = run(in_maps)
    # gather: out[b] = sum of the two head-half partials for b's batch pair
    outs = np.zeros((B, S, D), np.float32)
    for c in range(B):
        bp, hh = c % 4, c // 4
        part = np.asarray(results[c]["out"]).reshape(NB, S, D)
        for lb in range(NB):
            outs[2 * bp + lb] += part[lb]
    return outs, None


def kernel(**inputs) -> np.ndarray:
    out, _ = _run(inputs)
    return out


# revision 48
# speedup vs baseline: 1.0954x; 1.0954x over previous
"""DisentangledSelfAttention (DeBERTa-style) Trainium2 Bass kernel.

Sharding: 8 cores = 4 batch-pairs x 2 head-halves.  Core c handles batches
(2*(c%4), 2*(c%4)+1) and heads [6*(c//4), 6*(c//4)+6).  Each core emits the
partial output sum over its 6 heads for its 2 batches; the host adds the two
head-half partials per batch (standard tensor-parallel gather).  This halves
the replicated positional projections (Kp/Qp) versus pure batch parallelism.

Key algebraic structure exploited (same as the v1 kernel):
  rel[i, j] = j - i + 511 depends only on (j - i); for S=384 only rel rows
  128..894 (767 values) are used.  qp[i,p] = q[i].Kp[p] is bounced to DRAM
  [384x512] per (batch,head) and read back with row pitch 511 ("skew" read)
  which turns the per-row diagonal shift into a flat strided access; same for
  kq[j,p'] = k[j].Qp_rev[p'], read back transposed and accumulated into the
  score PSUM via identity matmuls.

Cost-model-driven choices:
  - All matmul operands are bf16 (1 cycle/row incl. transposes and short-N;
    halves every DMA transfer and enables DVE 2x/4x modes).  PSUM stays f32
    except the weight-transpose tiles (bf16 PSUM halves the copy-out cost).
  - softmax scale and q_bias are folded into Wq/bq on host; v_bias+bv are
    folded through Wo into the output bias on host (softmax rows sum to 1).
  - c2p is added into the score PSUM by an identity matmul (PE is cheaper
    than a DVE tensor_tensor on a 4-byte PSUM operand).
  - weight/activation uploads are single rearranged DMAs (HWDGE acquisition
    costs ~625ns per DMA instruction); qp+kq bounce scratch is one DRAM
    tensor per (batch, head) so the skew readback is a single fused DMA.
  - bounce DMAs run on the Pool/SWDGE queue (the SP/HWDGE path saturates on
    per-DMA issue cost), weight loads and output stores stay on SP.
  - PSUM->SBUF copies are split between DVE and Act; engines execute their
    queues in order, so the emission order doubles as a priority assignment.
  - PE also executes in order: the first four bounce-projection groups are
    emitted interleaved with the Kp/Qp projection chains so their copies
    drain while PE still has long chains to chew on, and the attention loop
    processes (batch,head) instances in PAIRS with tile-level interleave so
    an independent score/transpose group is always available during the
    exp->reciprocal->normalize latency chain.
"""

import os
import sys

import numpy as np

B, S, D, H = 8, 384, 768, 12
DH = D // H          # 64
MAX_POS = 512
NP = 767             # used relative positions (128..894)
SCALE = DH ** -0.5

NB = 2               # batches per core
NH = 6               # heads per core
DHALF = NH * DH      # 384 projection columns per core
NIT = S // 128       # 3 i/j tiles per batch
NKT = D // 128       # 6 contraction tiles over D
NOT = DHALF // 128   # 3 output tiles over the head half
NPP = 768            # positional axis padded to even
NW = 512             # per-i-tile window of the positional axis (511 used)
S2 = NB * S          # 768 tokens per core (2 batches)

_CACHE = {}


def _import_concourse():
    try:
        import concourse.bass  # noqa: F401
    except ImportError:
        for p in ("/opt/trn_rl_repo", "/root/.axon_site/_ro/trn_rl_repo"):
            if os.path.isdir(p) and p not in sys.path:
                sys.path.insert(0, p)
        import concourse.bass  # noqa: F401


def _build():
    """Build + finalize the per-core Bass program (identical on all cores)."""
    _import_concourse()
    import concourse.bass as bass
    import concourse.bacc as bacc
    import concourse.mybir as mybir
    import concourse.tile as tile
    from concourse.bass import ts
    from concourse.masks import make_identity
    from concourse.tile import add_dep_helper

    f32 = mybir.dt.float32
    bf16 = mybir.dt.bfloat16
    ADD = mybir.AluOpType.add
    EXP = mybir.ActivationFunctionType.Exp

    nc = bacc.Bacc("TRN2", target_bir_lowering=False, debug=False)

    # ---------------- DRAM I/O ----------------
    xT = nc.dram_tensor("xT", [D, S2], bf16, kind="ExternalInput")
    wq = nc.dram_tensor("wq", [D, DHALF], bf16, kind="ExternalInput")
    wk = nc.dram_tensor("wk", [D, DHALF], bf16, kind="ExternalInput")
    wv = nc.dram_tensor("wv", [D, DHALF], bf16, kind="ExternalInput")
    wpk = nc.dram_tensor("wpk", [D, DHALF], bf16, kind="ExternalInput")
    wpq = nc.dram_tensor("wpq", [D, DHALF], bf16, kind="ExternalInput")
    wo = nc.dram_tensor("wo", [DHALF, D], bf16, kind="ExternalInput")
    relkT = nc.dram_tensor("relkT", [D, NPP], bf16, kind="ExternalInput")
    bq = nc.dram_tensor("bq", [DHALF], f32, kind="ExternalInput")
    bk = nc.dram_tensor("bk", [DHALF], f32, kind="ExternalInput")
    bo = nc.dram_tensor("bo", [D], f32, kind="ExternalInput")
    out = nc.dram_tensor("out", [S2, D], f32, kind="ExternalOutput")

    NI = NB * NH     # 12 (batch, head) instances
    # one bounce scratch per instance: [0] = qp rows, [1] = kq rows
    bnc_dram = [
        nc.dram_tensor(f"bnc_scratch_{i}", [2, S, NW], bf16) for i in range(NI)
    ]

    with tile.TileContext(nc) as tc:
        with (
            tc.tile_pool(name="const", bufs=1) as constp,
            tc.tile_pool(name="big", bufs=1) as bigp,
            tc.tile_pool(name="wpool", bufs=3) as wpool,
            tc.tile_pool(name="work", bufs=3) as workp,
            tc.tile_pool(name="small", bufs=4) as smallp,
            tc.tile_pool(name="psA", bufs=3, space="PSUM") as psA,
            tc.tile_pool(name="psSC", bufs=3, space="PSUM") as psSC,
            tc.tile_pool(name="psWT", bufs=1, space="PSUM") as psWT,
            tc.tile_pool(name="psAV", bufs=1, space="PSUM") as psAV,
        ):
            import concourse.bass as bass_mod

            def psum(tag, shape=None, dtype=f32):
                pool = {"ps": psA, "sc": psSC, "wtps": psWT, "avps": psAV}[tag]
                return pool.tile(shape or [128, NW], dtype, tag=tag, name=tag)

            qT_sb = bigp.tile([128, NOT, S2], bf16, tag="qT")
            kT_sb = bigp.tile([128, NOT, S2], bf16, tag="kT")
            v_sb = bigp.tile([128, NB * NIT, DHALF], bf16, tag="v")
            KpT_sb = bigp.tile([128, NOT, NPP], bf16, tag="KpT")
            QpTr_sb = bigp.tile([128, NOT, NPP], bf16, tag="QpTr")
            attnT_sb = bigp.tile([128, NOT, S2], bf16, tag="attnT")

            bnc_w = [None] * NI
            bnc_sb = {}
            # GPSIMD cannot access PSUM, so PSUM->SBUF copies are split
            # between the DVE and Activation engines only.
            cp_engs = [nc.vector.tensor_copy, nc.scalar.copy]

            def inst_bh(i):
                return i // NH, i % NH

            def stage3_alloc(i):
                bnc_sb[i] = workp.tile(
                    [128, 2, NIT, NW], bf16, tag="bounce", bufs=4,
                    name="bounce",
                )
                bnc_w[i] = []

            def stage3_half(i, which):
                """qp (which=0) or kq (which=1) windowed matmuls + copies +
                bounce write for instance i.

                For i-tile t only positional columns [256-128t, 768-128t)
                are ever read back, so each row tile computes a 512-wide
                window; bounce rows are stored with pitch 512.  The two
                halves are emitted in separate pipeline slots so the copy
                burst injected into the DVE/Act queues stays short.
                """
                b, h = inst_bh(i)
                hp, ho = 64 * (h % 2), h // 2
                sb = bnc_sb[i]
                for it in range(NIT):
                    w0 = 256 - 128 * it
                    lhsT = (qT_sb if which == 0 else kT_sb)[
                        hp : hp + 64, ho, b * S + 128 * it : b * S + 128 * (it + 1)
                    ]
                    rhs = (KpT_sb if which == 0 else QpTr_sb)[
                        hp : hp + 64, ho, w0 : w0 + NW
                    ]
                    ps_t = psum("ps")
                    nc.tensor.matmul(ps_t[:], lhsT, rhs, start=True, stop=True)
                    idx = (2 * i + which) * 3 + it
                    cp_engs[1 if idx % 12 < 7 else 0](
                        sb[:, which, it, :], ps_t[:]
                    )
                bnc_w[i].append(nc.gpsimd.dma_start(
                    bnc_dram[i][which].rearrange("(o p) c -> p o c", p=128),
                    sb[:, which],
                ))

            def stage3(i):
                stage3_alloc(i)
                stage3_half(i, 0)
                stage3_half(i, 1)

            def skew_reads(i):
                """issue the fused skew readback for instance i.

                c2p[t][ip, jf] = qp[128t+ip, jf-ip+383-128t-w0off] and
                p2cT[t][u][jp, if] = kq[128u+jp, i-(128u+jp)+383]; flat
                addr within each half: 127 + 511*row + 65536*tile + col.
                """
                cp_sb = workp.tile([128, 2, NIT, S], bf16, tag="cp", bufs=4)
                r1 = nc.gpsimd.dma_start(
                    cp_sb[:],
                    bass_mod.AP(
                        bnc_dram[i], 127,
                        [[511, 128], [S * NW, 2], [128 * NW, NIT], [1, S]],
                    ),
                )
                for w_inst in bnc_w[i]:
                    add_dep_helper(r1.ins, w_inst.ins, reason="bounce rw")
                return cp_sb

            # ---------- stage 1+2: projections & positional projections ----
            with tc.tile_pool(name="bigtmp", bufs=2) as bigtmp:
                # DMA issue order is tuned for startup latency: the first
                # projection chain needs only wq + x(batch 0) halves, so
                # those transfers go first (transfers serialize on the DMA
                # engines).
                xT_sb = bigtmp.tile([128, NKT, S2], bf16, tag="bigtmp")
                wq_sb = wpool.tile([128, NKT, DHALF], bf16, tag="w")
                wk_sb = wpool.tile([128, NKT, DHALF], bf16, tag="w")
                nc.sync.dma_start(
                    wq_sb[:, 0:3, :],
                    wq[0 : D // 2].rearrange("(o p) c -> p o c", p=128),
                )
                nc.sync.dma_start(
                    xT_sb[:, 0:3, 0:S],
                    xT[0 : D // 2, 0:S].rearrange("(o p) c -> p o c", p=128),
                )
                nc.sync.dma_start(
                    wq_sb[:, 3:6, :],
                    wq[D // 2 : D].rearrange("(o p) c -> p o c", p=128),
                )
                nc.sync.dma_start(
                    xT_sb[:, 3:6, 0:S],
                    xT[D // 2 : D, 0:S].rearrange("(o p) c -> p o c", p=128),
                )
                nc.sync.dma_start(
                    wk_sb[:], wk[:].rearrange("(o p) c -> p o c", p=128)
                )
                nc.sync.dma_start(
                    xT_sb[:, :, S:S2],
                    xT[:, S:S2].rearrange("(o p) c -> p o c", p=128),
                )
                ident = constp.tile([128, 128], bf16, tag="ident")
                make_identity(nc, ident[:])

                bq_sb = constp.tile([128, NOT], f32, tag="bq")
                bk_sb = constp.tile([128, NOT], f32, tag="bk")
                bof = constp.tile([1, D], f32, tag="bo")
                bor = constp.tile([128, D], f32, tag="bor")
                nc.sync.dma_start(bq_sb[:], bq[:].rearrange("(o p) -> p o", p=128))
                nc.sync.dma_start(bk_sb[:], bk[:].rearrange("(o p) -> p o", p=128))
                nc.sync.dma_start(bof[:], bo[:].unsqueeze(0))
                nc.gpsimd.partition_broadcast(bor[:], bof[:])

                # q^T and k^T : [dout(part), i]  (bias per-partition)
                for w_sb, bias_sb, dst in (
                    (wq_sb, bq_sb, qT_sb), (wk_sb, bk_sb, kT_sb)
                ):
                    for b in range(NB):
                        for mo in range(NOT):
                            ps_t = psum("ps")
                            for ko in range(NKT):
                                nc.tensor.matmul(
                                    ps_t[:, :S],
                                    w_sb[:, ko, ts(mo, 128)],
                                    xT_sb[:, ko, b * S : (b + 1) * S],
                                    start=(ko == 0),
                                    stop=(ko == NKT - 1),
                                )
                            nc.vector.tensor_scalar_add(
                                dst[:, mo, b * S : (b + 1) * S],
                                ps_t[:, :S],
                                bias_sb[:, mo : mo + 1],
                            )

                # v : [j(part), dh]  (no bias: v_bias+bv folded into bo)
                wv_sb = wpool.tile([128, NKT, DHALF], bf16, tag="w")
                nc.sync.dma_start(
                    wv_sb[:], wv[:].rearrange("(o p) c -> p o c", p=128)
                )
                for b in range(NB):
                    for jt in range(NIT):
                        ps_t = psum("ps")
                        for ko in range(NKT):
                            nc.tensor.matmul(
                                ps_t[:, :DHALF],
                                xT_sb[:, ko, b * S + 128 * jt : b * S + 128 * (jt + 1)],
                                wv_sb[:, ko, :],
                                start=(ko == 0),
                                stop=(ko == NKT - 1),
                            )
                        nc.scalar.copy(
                            v_sb[:, NIT * b + jt, :], ps_t[:, :DHALF]
                        )

                # Kp^T and QpRev^T : [dout(part), p].  The reversed rel
                # operand is built on-chip from the forward copy (a DVE copy
                # with a negative-step AP).  Kp/Qp chunks are emitted
                # interleaved per mo-tile, and as soon as an mo-tile is
                # complete the bounce projections of the heads living in it
                # are emitted (their copies drain while PE runs the next
                # long projection chains).
                wpk_sb = wpool.tile([128, NKT, DHALF], bf16, tag="w")
                nc.sync.dma_start(
                    wpk_sb[:], wpk[:].rearrange("(o p) c -> p o c", p=128)
                )
                wpq_sb = wpool.tile([128, NKT, DHALF], bf16, tag="w",
                                    name="wpq_sb")
                nc.sync.dma_start(
                    wpq_sb[:], wpq[:].rearrange("(o p) c -> p o c", p=128)
                )
                rel_f = bigtmp.tile([128, NKT, NPP], bf16, tag="bigtmp",
                                    name="rel_f")
                nc.sync.dma_start(
                    rel_f[:], relkT[:].rearrange("(o p) c -> p o c", p=128)
                )
                rel_r = bigtmp.tile([128, NKT, NPP], bf16, tag="bigtmp",
                                    name="rel_r")
                # rel_rev[p'] = rel_fwd[766 - p']; col 767 copied from the
                # forward tile's zero pad
                nc.vector.tensor_copy(
                    rel_r[:, :, NPP - 1 : NPP], rel_f[:, :, NPP - 1 : NPP]
                )
                for ko in range(NKT):
                    fwd = rel_f[:, ko, 0 : NPP - 1]
                    rev = bass.AP(
                        fwd.tensor,
                        fwd.offset + (NPP - 2),
                        [[fwd.ap[0][0], 128], [-1, NPP - 1]],
                    )
                    nc.vector.tensor_copy(rel_r[:, ko, 0 : NPP - 1], rev)

                for mo in range(NOT):
                    for w_sb, rel_sb, dst in (
                        (wpk_sb, rel_f, KpT_sb), (wpq_sb, rel_r, QpTr_sb)
                    ):
                        for ci in range(2):
                            cs = 384 * ci
                            ps_t = psum("ps")
                            for ko in range(NKT):
                                nc.tensor.matmul(
                                    ps_t[:, :384],
                                    w_sb[:, ko, ts(mo, 128)],
                                    rel_sb[:, ko, cs : cs + 384],
                                    start=(ko == 0),
                                    stop=(ko == NKT - 1),
                                )
                            if (mo + ci) % 2 == 0:
                                nc.vector.tensor_copy(
                                    dst[:, mo, cs : cs + 384], ps_t[:, :384]
                                )
                            else:
                                nc.scalar.copy(
                                    dst[:, mo, cs : cs + 384], ps_t[:, :384]
                                )
                # prefetch Wo: fills the stage2->3 DMA lull
                wo_sb = wpool.tile([128, NOT, D], bf16, tag="w", name="wo_sb")
                nc.sync.dma_start(
                    wo_sb[:], wo[:].rearrange("(o p) c -> p o c", p=128)
                )

            # ---------- stages 4-6: attention per (batch, head) ------------
            def scores_tile(i, t, cp_sb, wT_sb):
                """scores + softmax + weight transpose for (instance, tile)."""
                b, h = inst_bh(i)
                hp, ho = 64 * (h % 2), h // 2
                # ---- scores psum: c2c + c2p (identity mm) + p2c^T
                sc_ps = psum("sc", shape=[128, S])
                nc.tensor.matmul(
                    sc_ps[:, :S],
                    qT_sb[hp : hp + 64, ho, b * S + 128 * t : b * S + 128 * (t + 1)],
                    kT_sb[hp : hp + 64, ho, b * S : (b + 1) * S],
                    start=True,
                    stop=False,
                    skip_group_check=True,
                )
                nc.tensor.matmul(
                    sc_ps[:, :S],
                    ident[:],
                    cp_sb[:, 0, t, :],
                    start=False,
                    stop=False,
                    skip_group_check=True,
                )
                for u in range(NIT):
                    # out[if, jf] = sum_jp p2ct[jp, if] * I[jp, jf]
                    nc.tensor.matmul(
                        sc_ps[:, ts(u, 128)],
                        cp_sb[:, 1, u, ts(t, 128)],
                        ident[:],
                        start=False,
                        stop=(u == NIT - 1),
                        skip_group_check=True,
                    )
                exp_sb = workp.tile([128, S], bf16, tag="exp", bufs=6)
                ssum = smallp.tile([128, 1], f32, tag="ssum")
                sinv = smallp.tile([128, 1], f32, tag="sinv")
                nc.scalar.activation(
                    exp_sb[:], sc_ps[:, :S], EXP, accum_out=ssum[:]
                )
                nc.vector.reciprocal(sinv[:], ssum[:])
                nc.vector.tensor_scalar_mul(exp_sb[:], exp_sb[:], sinv[:])
                # ---- transpose normalized weights -> wT[j, i]
                wt_ps = psum("wtps", shape=[128, S], dtype=bf16)
                for u in range(NIT):
                    nc.tensor.matmul(
                        wt_ps[:, ts(u, 128)],
                        exp_sb[:, ts(u, 128)],
                        ident[:],
                        is_transpose=True,
                        skip_group_check=True,
                    )
                cp_engs[0](
                    wT_sb[:, :, ts(t, 128)],
                    wt_ps[:].rearrange("p (u c) -> p u c", u=NIT),
                )

            def av(i, wT_sb):
                """stage 5: AV for one instance -> attnT."""
                b, h = inst_bh(i)
                hp, ho = 64 * (h % 2), h // 2
                av_ps = psum("avps", shape=[128, S])
                for u in range(NIT):
                    nc.tensor.matmul(
                        av_ps[0:64, :S],
                        v_sb[:, NIT * b + u, 64 * h : 64 * (h + 1)],
                        wT_sb[:, u, :],
                        start=(u == 0),
                        stop=(u == NIT - 1),
                    )
                cp_engs[0](
                    attnT_sb[hp : hp + 64, ho, b * S : (b + 1) * S],
                    av_ps[0:64, :S],
                )

            def outproj(b, it):
                """output projection for one 128-row tile of batch b."""
                o_sb = workp.tile([128, D], f32, tag="osb")
                for no in range(2):
                    ps_t = psum("ps")
                    for ko in range(NOT):
                        nc.tensor.matmul(
                            ps_t[:, :384],
                            attnT_sb[:, ko, b * S + 128 * it : b * S + 128 * (it + 1)],
                            wo_sb[:, ko, ts(no, 384)],
                            start=(ko == 0),
                            stop=(ko == NOT - 1),
                        )
                    nc.vector.tensor_tensor(
                        o_sb[:, ts(no, 384)], ps_t[:, :384],
                        bor[:, ts(no, 384)], ADD,
                    )
                nc.sync.dma_start(
                    out[b * S + 128 * it : b * S + 128 * (it + 1), :],
                    o_sb[:],
                )

            # software pipeline over instance PAIRS with tile interleave.
            # Instances 0..3 already have bounces in flight (emitted during
            # stage 2); each pair keeps two more instances' bounce halves
            # moving through the spare pipeline slots, and batch 0's output
            # projection fills the slots where no stage3 work remains.
            stage3(0)
            stage3(1)
            for p in range(NI // 2):
                i, j = 2 * p, 2 * p + 1
                wti = workp.tile([128, NIT, S], bf16, tag="wT", bufs=4,
                                 name=f"wT{i}")
                wtj = workp.tile([128, NIT, S], bf16, tag="wT", bufs=4,
                                 name=f"wT{j}")
                cpi = skew_reads(i)
                cpj = skew_reads(j)
                for t in range(NIT):
                    scores_tile(i, t, cpi, wti)
                    if t == 0 and 2 * p + 2 < NI:
                        stage3_alloc(2 * p + 2)
                        stage3_half(2 * p + 2, 0)
                    if t == 1 and 2 * p + 3 < NI:
                        stage3_alloc(2 * p + 3)
                        stage3_half(2 * p + 3, 0)
                    scores_tile(j, t, cpj, wtj)
                    if t == 0 and 2 * p + 2 < NI:
                        stage3_half(2 * p + 2, 1)
                    if t == 1 and 2 * p + 3 < NI:
                        stage3_half(2 * p + 3, 1)
                    if p == 4 and t == 2:
                        outproj(0, 0)
                    if p == 5 and t in (0, 1):
                        outproj(0, t + 1)
                av(i, wti)
                av(j, wtj)
            for it in range(NIT):
                outproj(1, it)

    nc.finalize()
    return nc


def _get_program():
    if "nc" not in _CACHE:
        _CACHE["nc"] = _build()
    return _CACHE["nc"]


def _host_prep(inputs):
    import ml_dtypes

    f = np.float32
    bf = ml_dtypes.bfloat16
    x = np.asarray(inputs["x"], f)
    rel = np.asarray(inputs["rel_pos_emb"], f)
    rel_used = rel[MAX_POS - S : MAX_POS - S + NP]          # rows 128..894
    relkT = np.ascontiguousarray(
        np.pad(rel_used.T, ((0, 0), (0, 1)))).astype(bf)

    Wq = np.asarray(inputs["Wq"], f) * SCALE
    Wk = np.asarray(inputs["Wk"], f)
    Wv = np.asarray(inputs["Wv"], f)
    Wpk = np.asarray(inputs["Wpk"], f)
    Wpq = np.asarray(inputs["Wpq"], f) * SCALE
    Wo = np.asarray(inputs["Wo"], f)
    bq_full = (np.asarray(inputs["bq"], f) + np.asarray(inputs["q_bias"], f)) * SCALE
    bk_full = np.asarray(inputs["bk"], f)
    bvv = np.asarray(inputs["bv"], f) + np.asarray(inputs["v_bias"], f)
    bo_full = np.asarray(inputs["bo"], f)

    in_maps = []
    for c in range(B):
        bp, hh = c % 4, c // 4
        hs = slice(hh * DHALF, (hh + 1) * DHALF)
        b0, b1 = 2 * bp, 2 * bp + 1
        xT = np.concatenate([x[b0].T, x[b1].T], axis=1)
        # v_bias+bv pass through attention unchanged (softmax rows sum to 1)
        # and then through this core's half of Wo; bo itself added once (hh=0).
        bo_c = bvv[hs] @ Wo[hs] + (bo_full if hh == 0 else 0.0)
        in_maps.append({
            "xT": np.ascontiguousarray(xT).astype(bf),
            "wq": np.ascontiguousarray(Wq[:, hs]).astype(bf),
            "wk": np.ascontiguousarray(Wk[:, hs]).astype(bf),
            "wv": np.ascontiguousarray(Wv[:, hs]).astype(bf),
            "wpk": np.ascontiguousarray(Wpk[:, hs]).astype(bf),
            "wpq": np.ascontiguousarray(Wpq[:, hs]).astype(bf),
            "wo": np.ascontiguousarray(Wo[hs]).astype(bf),
            "relkT": relkT,
            "bq": np.ascontiguousarray(bq_full[hs]),
            "bk": np.ascontiguousarray(bk_full[hs]),
            "bo": bo_c.astype(f),
        })
    return in_maps


def _get_runner():
    """Build (once) a jitted SPMD executor for the compiled program."""
    key = "runner"
    if key in _CACHE:
        return _CACHE[key]
    _import_concourse()
    import jax
    from jax.sharding import Mesh, PartitionSpec
    from jax.experimental.shard_map import shard_map
    import concourse.mybir as mybir
    from concourse import bass2jax

    nc = _get_program()
    bass2jax.install_neuronx_cc_hook()

    partition_name = (
        nc.partition_id_tensor.name if nc.partition_id_tensor else None
    )
    in_names, out_names, out_avals, zero_outs = [], [], [], []
    for alloc in nc.m.functions[0].allocations:
        if not isinstance(alloc, mybir.MemoryLocationSet):
            continue
        name = alloc.memorylocations[0].name
        if alloc.kind == "ExternalInput":
            if name != partition_name:
                in_names.append(name)
        elif alloc.kind == "ExternalOutput":
            out_names.append(name)
            shape = tuple(alloc.tensor_shape)
            dtype = mybir.dt.np(alloc.dtype)
            out_avals.append(jax.core.ShapedArray(shape, dtype))
            zero_outs.append(np.zeros(shape, dtype))
    n_params = len(in_names)
    all_names = in_names + out_names
    if partition_name is not None:
        all_names = all_names + [partition_name]

    def _body(*args):
        operands = list(args)
        if partition_name is not None:
            operands.append(bass2jax.partition_id_tensor())
        outs = bass2jax._bass_exec_p.bind(
            *operands,
            out_avals=tuple(out_avals),
            in_names=tuple(all_names),
            out_names=tuple(out_names),
            lowering_input_output_aliases=(),
            sim_require_finite=True,
            sim_require_nnan=True,
            nc=nc,
        )
        return tuple(outs)

    devices = jax.devices()[:B]
    mesh = Mesh(np.asarray(devices), ("core",))
    n_outs = len(out_names)
    sharded = jax.jit(
        shard_map(
            _body,
            mesh=mesh,
            in_specs=(PartitionSpec("core"),) * (n_params + n_outs),
            out_specs=(PartitionSpec("core"),) * n_outs,
            check_rep=False,
        ),
        donate_argnums=tuple(range(n_params, n_params + n_outs)),
        keep_unused=True,
    )

    def run(in_maps):
        concat_in = [
            np.concatenate([np.asarray(in_maps[c][nm]) for c in range(B)], axis=0)
            for nm in in_names
        ]
        concat_zeros = [
            np.zeros((B * z.shape[0], *z.shape[1:]), z.dtype) for z in zero_outs
        ]
        out_arrs = sharded(*concat_in, *concat_zeros)
        return [
            {
                nm: np.asarray(out_arrs[i]).reshape(B, *out_avals[i].shape)[c]
                for i, nm in enumerate(out_names)
            }
            for c in range(B)
        ]

    _CACHE[key] = run
    return run


def _run(inputs, trace=False):
    run = _get_runner()
    in_maps = _host_prep(inputs)
    results = run(in_maps)
    # gather: out[b] = sum of the two head-half partials for b's batch pair
    outs = np.zeros((B, S, D), np.float32)
    for c in range(B):
        bp, hh = c % 4, c // 4
        part = np.asarray(results[c]["out"]).reshape(NB, S, D)
        for lb in range(NB):
            outs[2 * bp + lb] += part[lb]
    return outs, None


def kernel(**inputs) -> np.ndarray:
    out, _ = _run(inputs)
    return out


# revision 66
# speedup vs baseline: 1.0986x; 1.0029x over previous
"""DisentangledSelfAttention (DeBERTa-style) Trainium2 Bass kernel.

Sharding: 8 cores = 4 batch-pairs x 2 head-halves.  Core c handles batches
(2*(c%4), 2*(c%4)+1) and heads [6*(c//4), 6*(c//4)+6).  Each core emits the
partial output sum over its 6 heads for its 2 batches; the host adds the two
head-half partials per batch (standard tensor-parallel gather).  This halves
the replicated positional projections (Kp/Qp) versus pure batch parallelism.

Key algebraic structure exploited (same as the v1 kernel):
  rel[i, j] = j - i + 511 depends only on (j - i); for S=384 only rel rows
  128..894 (767 values) are used.  qp[i,p] = q[i].Kp[p] is bounced to DRAM
  [384x512] per (batch,head) and read back with row pitch 511 ("skew" read)
  which turns the per-row diagonal shift into a flat strided access; same for
  kq[j,p'] = k[j].Qp_rev[p'], read back transposed and accumulated into the
  score PSUM via identity matmuls.

Cost-model-driven choices:
  - All matmul operands are bf16 (1 cycle/row incl. transposes and short-N;
    halves every DMA transfer and enables DVE 2x/4x modes).  PSUM stays f32
    except the weight-transpose tiles (bf16 PSUM halves the copy-out cost).
  - softmax scale and q_bias are folded into Wq/bq on host; v_bias+bv are
    folded through Wo into the output bias on host (softmax rows sum to 1).
  - c2p is added into the score PSUM by an identity matmul (PE is cheaper
    than a DVE tensor_tensor on a 4-byte PSUM operand).
  - weight/activation uploads are single rearranged DMAs (HWDGE acquisition
    costs ~625ns per DMA instruction); qp+kq bounce scratch is one DRAM
    tensor per (batch, head) so the skew readback is a single fused DMA.
  - bounce DMAs run on the Pool/SWDGE queue (the SP/HWDGE path saturates on
    per-DMA issue cost), weight loads and output stores stay on SP.
  - PSUM->SBUF copies are split between DVE and Act; engines execute their
    queues in order, so the emission order doubles as a priority assignment.
  - PE also executes in order: the first four bounce-projection groups are
    emitted interleaved with the Kp/Qp projection chains so their copies
    drain while PE still has long chains to chew on, and the attention loop
    processes (batch,head) instances in PAIRS with tile-level interleave so
    an independent score/transpose group is always available during the
    exp->reciprocal->normalize latency chain.
"""

import os
import sys

import numpy as np

B, S, D, H = 8, 384, 768, 12
DH = D // H          # 64
MAX_POS = 512
NP = 767             # used relative positions (128..894)
SCALE = DH ** -0.5

NB = 2               # batches per core
NH = 6               # heads per core
DHALF = NH * DH      # 384 projection columns per core
NIT = S // 128       # 3 i/j tiles per batch
NKT = D // 128       # 6 contraction tiles over D
NOT = DHALF // 128   # 3 output tiles over the head half
NPP = 768            # positional axis padded to even
NW = 512             # per-i-tile window of the positional axis (511 used)
S2 = NB * S          # 768 tokens per core (2 batches)

_CACHE = {}


def _import_concourse():
    try:
        import concourse.bass  # noqa: F401
    except ImportError:
        for p in ("/opt/trn_rl_repo", "/root/.axon_site/_ro/trn_rl_repo"):
            if os.path.isdir(p) and p not in sys.path:
                sys.path.insert(0, p)
        import concourse.bass  # noqa: F401


def _build():
    """Build + finalize the per-core Bass program (identical on all cores)."""
    _import_concourse()
    import concourse.bass as bass
    import concourse.bacc as bacc
    import concourse.mybir as mybir
    import concourse.tile as tile
    from concourse.bass import ts
    from concourse.masks import make_identity
    from concourse.tile import add_dep_helper

    f32 = mybir.dt.float32
    bf16 = mybir.dt.bfloat16
    ADD = mybir.AluOpType.add
    EXP = mybir.ActivationFunctionType.Exp

    nc = bacc.Bacc("TRN2", target_bir_lowering=False, debug=False)

    # ---------------- DRAM I/O ----------------
    xT = nc.dram_tensor("xT", [D, S2], bf16, kind="ExternalInput")
    wq = nc.dram_tensor("wq", [D, DHALF], bf16, kind="ExternalInput")
    wk = nc.dram_tensor("wk", [D, DHALF], bf16, kind="ExternalInput")
    wv = nc.dram_tensor("wv", [D, DHALF], bf16, kind="ExternalInput")
    wpk = nc.dram_tensor("wpk", [D, DHALF], bf16, kind="ExternalInput")
    wpq = nc.dram_tensor("wpq", [D, DHALF], bf16, kind="ExternalInput")
    wo = nc.dram_tensor("wo", [DHALF, D], bf16, kind="ExternalInput")
    relkT = nc.dram_tensor("relkT", [D, NPP], bf16, kind="ExternalInput")
    bq = nc.dram_tensor("bq", [DHALF], f32, kind="ExternalInput")
    bk = nc.dram_tensor("bk", [DHALF], f32, kind="ExternalInput")
    bo = nc.dram_tensor("bo", [D], f32, kind="ExternalInput")
    out = nc.dram_tensor("out", [S2, D], f32, kind="ExternalOutput")

    NI = NB * NH     # 12 (batch, head) instances
    # one bounce scratch per instance: [0] = qp rows, [1] = kq rows
    bnc_dram = [
        nc.dram_tensor(f"bnc_scratch_{i}", [2, S, NW], bf16) for i in range(NI)
    ]

    with tile.TileContext(nc) as tc:
        with (
            tc.tile_pool(name="const", bufs=1) as constp,
            tc.tile_pool(name="big", bufs=1) as bigp,
            tc.tile_pool(name="wpool", bufs=3) as wpool,
            tc.tile_pool(name="work", bufs=3) as workp,
            tc.tile_pool(name="small", bufs=4) as smallp,
            tc.tile_pool(name="psA", bufs=3, space="PSUM") as psA,
            tc.tile_pool(name="psSC", bufs=3, space="PSUM") as psSC,
            tc.tile_pool(name="psWT", bufs=1, space="PSUM") as psWT,
            tc.tile_pool(name="psAV", bufs=1, space="PSUM") as psAV,
        ):
            import concourse.bass as bass_mod

            def psum(tag, shape=None, dtype=f32):
                pool = {"ps": psA, "sc": psSC, "wtps": psWT, "avps": psAV}[tag]
                return pool.tile(shape or [128, NW], dtype, tag=tag, name=tag)

            qT_sb = bigp.tile([128, NOT, S2], bf16, tag="qT")
            kT_sb = bigp.tile([128, NOT, S2], bf16, tag="kT")
            v_sb = bigp.tile([128, NB * NIT, DHALF], bf16, tag="v")
            KpT_sb = bigp.tile([128, NOT, NPP], bf16, tag="KpT")
            QpTr_sb = bigp.tile([128, NOT, NPP], bf16, tag="QpTr")
            attnT_sb = bigp.tile([128, NOT, S2], bf16, tag="attnT")

            bnc_w = [None] * NI
            bnc_sb = {}
            # GPSIMD cannot access PSUM, so PSUM->SBUF copies are split
            # between the DVE and Activation engines only.
            cp_engs = [nc.vector.tensor_copy, nc.scalar.copy]

            def inst_bh(i):
                return i // NH, i % NH

            def stage3_alloc(i):
                bnc_sb[i] = workp.tile(
                    [128, 2, NIT, NW], bf16, tag="bounce", bufs=4,
                    name="bounce",
                )
                bnc_w[i] = []

            def stage3_half(i, which):
                """qp (which=0) or kq (which=1) windowed matmuls + copies +
                bounce write for instance i.

                For i-tile t only positional columns [256-128t, 768-128t)
                are ever read back, so each row tile computes a 512-wide
                window; bounce rows are stored with pitch 512.  The two
                halves are emitted in separate pipeline slots so the copy
                burst injected into the DVE/Act queues stays short.
                """
                b, h = inst_bh(i)
                hp, ho = 64 * (h % 2), h // 2
                sb = bnc_sb[i]
                for it in range(NIT):
                    w0 = 256 - 128 * it
                    lhsT = (qT_sb if which == 0 else kT_sb)[
                        hp : hp + 64, ho, b * S + 128 * it : b * S + 128 * (it + 1)
                    ]
                    rhs = (KpT_sb if which == 0 else QpTr_sb)[
                        hp : hp + 64, ho, w0 : w0 + NW
                    ]
                    ps_t = psum("ps")
                    nc.tensor.matmul(ps_t[:], lhsT, rhs, start=True, stop=True)
                    idx = (2 * i + which) * 3 + it
                    cp_engs[1 if idx % 12 < 7 else 0](
                        sb[:, which, it, :], ps_t[:]
                    )
                bnc_w[i].append(nc.gpsimd.dma_start(
                    bnc_dram[i][which].rearrange("(o p) c -> p o c", p=128),
                    sb[:, which],
                ))

            def stage3(i):
                stage3_alloc(i)
                stage3_half(i, 0)
                stage3_half(i, 1)

            def skew_reads(i):
                """issue the fused skew readback for instance i.

                c2p[t][ip, jf] = qp[128t+ip, jf-ip+383-128t-w0off] and
                p2cT[t][u][jp, if] = kq[128u+jp, i-(128u+jp)+383]; flat
                addr within each half: 127 + 511*row + 65536*tile + col.
                """
                cp_sb = workp.tile([128, 2, NIT, S], bf16, tag="cp", bufs=4)
                r1 = nc.gpsimd.dma_start(
                    cp_sb[:],
                    bass_mod.AP(
                        bnc_dram[i], 127,
                        [[511, 128], [S * NW, 2], [128 * NW, NIT], [1, S]],
                    ),
                )
                for w_inst in bnc_w[i]:
                    add_dep_helper(r1.ins, w_inst.ins, reason="bounce rw")
                return cp_sb

            # ---------- stage 1+2: projections & positional projections ----
            with tc.tile_pool(name="bigtmp", bufs=2) as bigtmp:
                # DMA issue order is tuned for startup latency: the first
                # projection chain needs only wq + x(batch 0) halves, so
                # those transfers go first (transfers serialize on the DMA
                # engines).
                xT_sb = bigtmp.tile([128, NKT, S2], bf16, tag="bigtmp")
                wq_sb = wpool.tile([128, NKT, DHALF], bf16, tag="w")
                wk_sb = wpool.tile([128, NKT, DHALF], bf16, tag="w")
                nc.sync.dma_start(
                    wq_sb[:, 0:3, :],
                    wq[0 : D // 2].rearrange("(o p) c -> p o c", p=128),
                )
                nc.sync.dma_start(
                    xT_sb[:, 0:3, 0:S],
                    xT[0 : D // 2, 0:S].rearrange("(o p) c -> p o c", p=128),
                )
                nc.sync.dma_start(
                    wq_sb[:, 3:6, :],
                    wq[D // 2 : D].rearrange("(o p) c -> p o c", p=128),
                )
                nc.sync.dma_start(
                    xT_sb[:, 3:6, 0:S],
                    xT[D // 2 : D, 0:S].rearrange("(o p) c -> p o c", p=128),
                )
                nc.sync.dma_start(
                    wk_sb[:], wk[:].rearrange("(o p) c -> p o c", p=128)
                )
                nc.sync.dma_start(
                    xT_sb[:, :, S:S2],
                    xT[:, S:S2].rearrange("(o p) c -> p o c", p=128),
                )
                ident = constp.tile([128, 128], bf16, tag="ident")
                make_identity(nc, ident[:])

                bq_sb = constp.tile([128, NOT], f32, tag="bq")
                bk_sb = constp.tile([128, NOT], f32, tag="bk")
                bof = constp.tile([1, D], f32, tag="bo")
                bor = constp.tile([128, D], f32, tag="bor")
                nc.sync.dma_start(bq_sb[:], bq[:].rearrange("(o p) -> p o", p=128))
                nc.sync.dma_start(bk_sb[:], bk[:].rearrange("(o p) -> p o", p=128))
                nc.sync.dma_start(bof[:], bo[:].unsqueeze(0))
                nc.gpsimd.partition_broadcast(bor[:], bof[:])

                # q^T and k^T : [dout(part), i]  (bias per-partition)
                for w_sb, bias_sb, dst in (
                    (wq_sb, bq_sb, qT_sb), (wk_sb, bk_sb, kT_sb)
                ):
                    for b in range(NB):
                        for mo in range(NOT):
                            ps_t = psum("ps")
                            for ko in range(NKT):
                                nc.tensor.matmul(
                                    ps_t[:, :S],
                                    w_sb[:, ko, ts(mo, 128)],
                                    xT_sb[:, ko, b * S : (b + 1) * S],
                                    start=(ko == 0),
                                    stop=(ko == NKT - 1),
                                )
                            nc.vector.tensor_scalar_add(
                                dst[:, mo, b * S : (b + 1) * S],
                                ps_t[:, :S],
                                bias_sb[:, mo : mo + 1],
                            )

                # v : [j(part), dh]  (no bias: v_bias+bv folded into bo)
                wv_sb = wpool.tile([128, NKT, DHALF], bf16, tag="w")
                nc.sync.dma_start(
                    wv_sb[:], wv[:].rearrange("(o p) c -> p o c", p=128)
                )
                for b in range(NB):
                    for jt in range(NIT):
                        ps_t = psum("ps")
                        for ko in range(NKT):
                            nc.tensor.matmul(
                                ps_t[:, :DHALF],
                                xT_sb[:, ko, b * S + 128 * jt : b * S + 128 * (jt + 1)],
                                wv_sb[:, ko, :],
                                start=(ko == 0),
                                stop=(ko == NKT - 1),
                            )
                        nc.scalar.copy(
                            v_sb[:, NIT * b + jt, :], ps_t[:, :DHALF]
                        )

                # Kp^T and QpRev^T : [dout(part), p].  The reversed rel
                # operand is built on-chip from the forward copy (a DVE copy
                # with a negative-step AP).  Kp/Qp chunks are emitted
                # interleaved per mo-tile, and as soon as an mo-tile is
                # complete the bounce projections of the heads living in it
                # are emitted (their copies drain while PE runs the next
                # long projection chains).
                wpk_sb = wpool.tile([128, NKT, DHALF], bf16, tag="w")
                nc.sync.dma_start(
                    wpk_sb[:], wpk[:].rearrange("(o p) c -> p o c", p=128)
                )
                wpq_sb = wpool.tile([128, NKT, DHALF], bf16, tag="w",
                                    name="wpq_sb")
                nc.sync.dma_start(
                    wpq_sb[:], wpq[:].rearrange("(o p) c -> p o c", p=128)
                )
                rel_f = bigtmp.tile([128, NKT, NPP], bf16, tag="bigtmp",
                                    name="rel_f")
                nc.sync.dma_start(
                    rel_f[:], relkT[:].rearrange("(o p) c -> p o c", p=128)
                )
                rel_r = bigtmp.tile([128, NKT, NPP], bf16, tag="bigtmp",
                                    name="rel_r")
                # rel_rev[p'] = rel_fwd[766 - p']; col 767 copied from the
                # forward tile's zero pad
                nc.vector.tensor_copy(
                    rel_r[:, :, NPP - 1 : NPP], rel_f[:, :, NPP - 1 : NPP]
                )
                for ko in range(NKT):
                    fwd = rel_f[:, ko, 0 : NPP - 1]
                    rev = bass.AP(
                        fwd.tensor,
                        fwd.offset + (NPP - 2),
                        [[fwd.ap[0][0], 128], [-1, NPP - 1]],
                    )
                    nc.vector.tensor_copy(rel_r[:, ko, 0 : NPP - 1], rev)

                for mo in range(NOT):
                    for w_sb, rel_sb, dst in (
                        (wpk_sb, rel_f, KpT_sb), (wpq_sb, rel_r, QpTr_sb)
                    ):
                        for ci in range(2):
                            cs = 384 * ci
                            ps_t = psum("ps")
                            for ko in range(NKT):
                                nc.tensor.matmul(
                                    ps_t[:, :384],
                                    w_sb[:, ko, ts(mo, 128)],
                                    rel_sb[:, ko, cs : cs + 384],
                                    start=(ko == 0),
                                    stop=(ko == NKT - 1),
                                )
                            nc.scalar.copy(
                                dst[:, mo, cs : cs + 384], ps_t[:, :384]
                            )
                # prefetch Wo: fills the stage2->3 DMA lull
                wo_sb = wpool.tile([128, NOT, D], bf16, tag="w", name="wo_sb")
                nc.sync.dma_start(
                    wo_sb[:], wo[:].rearrange("(o p) c -> p o c", p=128)
                )

            # ---------- stages 4-6: attention per (batch, head) ------------
            def scores_tile(i, t, cp_sb, wT_sb):
                """scores + softmax for (instance, tile); the weight
                transpose is emitted one pipeline slot later (PE executes in
                order, so a transpose waiting on the softmax chain would
                head-of-line block the next ready score group)."""
                b, h = inst_bh(i)
                hp, ho = 64 * (h % 2), h // 2
                # ---- scores psum: c2c + c2p (identity mm) + p2c^T
                sc_ps = psum("sc", shape=[128, S])
                nc.tensor.matmul(
                    sc_ps[:, :S],
                    qT_sb[hp : hp + 64, ho, b * S + 128 * t : b * S + 128 * (t + 1)],
                    kT_sb[hp : hp + 64, ho, b * S : (b + 1) * S],
                    start=True,
                    stop=False,
                    skip_group_check=True,
                )
                nc.tensor.matmul(
                    sc_ps[:, :S],
                    ident[:],
                    cp_sb[:, 0, t, :],
                    start=False,
                    stop=False,
                    skip_group_check=True,
                )
                for u in range(NIT):
                    # out[if, jf] = sum_jp p2ct[jp, if] * I[jp, jf]
                    nc.tensor.matmul(
                        sc_ps[:, ts(u, 128)],
                        cp_sb[:, 1, u, ts(t, 128)],
                        ident[:],
                        start=False,
                        stop=(u == NIT - 1),
                        skip_group_check=True,
                    )
                exp_sb = workp.tile([128, S], bf16, tag="exp", bufs=6)
                ssum = smallp.tile([128, 1], f32, tag="ssum")
                sinv = smallp.tile([128, 1], f32, tag="sinv")
                nc.scalar.activation(
                    exp_sb[:], sc_ps[:, :S], EXP, accum_out=ssum[:]
                )
                nc.vector.reciprocal(sinv[:], ssum[:])
                nc.vector.tensor_scalar_mul(exp_sb[:], exp_sb[:], sinv[:])
                return exp_sb

            def transpose_tile(t, exp_sb, wT_sb):
                """transpose normalized weights -> wT[j, i] for one tile."""
                wt_ps = psum("wtps", shape=[128, S], dtype=bf16)
                for u in range(NIT):
                    nc.tensor.matmul(
                        wt_ps[:, ts(u, 128)],
                        exp_sb[:, ts(u, 128)],
                        ident[:],
                        is_transpose=True,
                        skip_group_check=True,
                    )
                cp_engs[0](
                    wT_sb[:, :, ts(t, 128)],
                    wt_ps[:].rearrange("p (u c) -> p u c", u=NIT),
                )

            def av(i, wT_sb):
                """stage 5: AV for one instance -> attnT."""
                b, h = inst_bh(i)
                hp, ho = 64 * (h % 2), h // 2
                av_ps = psum("avps", shape=[128, S])
                for u in range(NIT):
                    nc.tensor.matmul(
                        av_ps[0:64, :S],
                        v_sb[:, NIT * b + u, 64 * h : 64 * (h + 1)],
                        wT_sb[:, u, :],
                        start=(u == 0),
                        stop=(u == NIT - 1),
                    )
                cp_engs[0](
                    attnT_sb[hp : hp + 64, ho, b * S : (b + 1) * S],
                    av_ps[0:64, :S],
                )

            def outproj(b, it):
                """output projection for one 128-row tile of batch b.

                The output bias is accumulated into the PSUM by a K=1
                ones-row matmul, so the PSUM->SBUF move is a plain copy that
                can run on either engine, and each 384-wide half is stored
                as soon as its copy lands.
                """
                o_sb = workp.tile([128, D], f32, tag="osb")
                for no in range(2):
                    ps_t = psum("ps")
                    for ko in range(NOT):
                        nc.tensor.matmul(
                            ps_t[:, :384],
                            attnT_sb[:, ko, b * S + 128 * it : b * S + 128 * (it + 1)],
                            wo_sb[:, ko, ts(no, 384)],
                            start=(ko == 0),
                            stop=(ko == NOT - 1),
                        )
                    nc.vector.tensor_tensor(
                        o_sb[:, ts(no, 384)], ps_t[:, :384],
                        bor[:, ts(no, 384)], ADD,
                    )
                nc.sync.dma_start(
                    out[b * S + 128 * it : b * S + 128 * (it + 1), :],
                    o_sb[:],
                )

            # software pipeline over instance PAIRS with tile interleave.
            # Instances 0..3 already have bounces in flight (emitted during
            # stage 2); each pair keeps two more instances' bounce halves
            # moving through the spare pipeline slots, and batch 0's output
            # projection fills the slots where no stage3 work remains.
            # software pipeline over instance PAIRS with tile interleave:
            # while one instance's exp->reciprocal->normalize chain runs,
            # the other instance's score/transpose groups keep PE busy.
            # Weight transposes trail their softmax by roughly one slot so
            # they never head-of-line block the in-order PE queue.
            stage3(0)
            stage3(1)
            for p in range(NI // 2):
                i, j = 2 * p, 2 * p + 1
                wti = workp.tile([128, NIT, S], bf16, tag="wT", bufs=4,
                                 name=f"wT{i}")
                wtj = workp.tile([128, NIT, S], bf16, tag="wT", bufs=4,
                                 name=f"wT{j}")
                cpi = skew_reads(i)
                cpj = skew_reads(j)
                expi = [None] * NIT
                expj = [None] * NIT
                for t in range(NIT):
                    expi[t] = scores_tile(i, t, cpi, wti)
                    if t >= 1:
                        transpose_tile(t - 1, expj[t - 1], wtj)
                    if t == 0 and 2 * p + 2 < NI:
                        stage3_alloc(2 * p + 2)
                        stage3_half(2 * p + 2, 0)
                    if t == 1 and 2 * p + 3 < NI:
                        stage3_alloc(2 * p + 3)
                        stage3_half(2 * p + 3, 0)
                    expj[t] = scores_tile(j, t, cpj, wtj)
                    transpose_tile(t, expi[t], wti)
                    if t == 0 and 2 * p + 2 < NI:
                        stage3_half(2 * p + 2, 1)
                    if t == 1 and 2 * p + 3 < NI:
                        stage3_half(2 * p + 3, 1)
                    if p == 4 and t == 2:
                        outproj(0, 0)
                    if p == 5 and t in (0, 1):
                        outproj(0, t + 1)
                transpose_tile(NIT - 1, expj[NIT - 1], wtj)
                av(i, wti)
                av(j, wtj)
            for it in range(NIT):
                outproj(1, it)

    nc.finalize()
    return nc


def _get_program():
    if "nc" not in _CACHE:
        _CACHE["nc"] = _build()
    return _CACHE["nc"]


def _host_prep(inputs):
    import ml_dtypes

    f = np.float32
    bf = ml_dtypes.bfloat16
    x = np.asarray(inputs["x"], f)
    rel = np.asarray(inputs["rel_pos_emb"], f)
    rel_used = rel[MAX_POS - S : MAX_POS - S + NP]          # rows 128..894
    relkT = np.ascontiguousarray(
        np.pad(rel_used.T, ((0, 0), (0, 1)))).astype(bf)

    Wq = np.asarray(inputs["Wq"], f) * SCALE
    Wk = np.asarray(inputs["Wk"], f)
    Wv = np.asarray(inputs["Wv"], f)
    Wpk = np.asarray(inputs["Wpk"], f)
    Wpq = np.asarray(inputs["Wpq"], f) * SCALE
    Wo = np.asarray(inputs["Wo"], f)
    bq_full = (np.asarray(inputs["bq"], f) + np.asarray(inputs["q_bias"], f)) * SCALE
    bk_full = np.asarray(inputs["bk"], f)
    bvv = np.asarray(inputs["bv"], f) + np.asarray(inputs["v_bias"], f)
    bo_full = np.asarray(inputs["bo"], f)

    in_maps = []
    for c in range(B):
        bp, hh = c % 4, c // 4
        hs = slice(hh * DHALF, (hh + 1) * DHALF)
        b0, b1 = 2 * bp, 2 * bp + 1
        xT = np.concatenate([x[b0].T, x[b1].T], axis=1)
        # v_bias+bv pass through attention unchanged (softmax rows sum to 1)
        # and then through this core's half of Wo; bo itself added once (hh=0).
        bo_c = bvv[hs] @ Wo[hs] + (bo_full if hh == 0 else 0.0)
        in_maps.append({
            "xT": np.ascontiguousarray(xT).astype(bf),
            "wq": np.ascontiguousarray(Wq[:, hs]).astype(bf),
            "wk": np.ascontiguousarray(Wk[:, hs]).astype(bf),
            "wv": np.ascontiguousarray(Wv[:, hs]).astype(bf),
            "wpk": np.ascontiguousarray(Wpk[:, hs]).astype(bf),
            "wpq": np.ascontiguousarray(Wpq[:, hs]).astype(bf),
            "wo": np.ascontiguousarray(Wo[hs]).astype(bf),
            "relkT": relkT,
            "bq": np.ascontiguousarray(bq_full[hs]),
            "bk": np.ascontiguousarray(bk_full[hs]),
            "bo": bo_c.astype(f),
        })
    return in_maps


def _get_runner():
    """Build (once) a jitted SPMD executor for the compiled program."""
    key = "runner"
    if key in _CACHE:
        return _CACHE[key]
    _import_concourse()
    import jax
    from jax.sharding import Mesh, PartitionSpec
    from jax.experimental.shard_map import shard_map
    import concourse.mybir as mybir
    from concourse import bass2jax

    nc = _get_program()
    bass2jax.install_neuronx_cc_hook()

    partition_name = (
        nc.partition_id_tensor.name if nc.partition_id_tensor else None
    )
    in_names, out_names, out_avals, zero_outs = [], [], [], []
    for alloc in nc.m.functions[0].allocations:
        if not isinstance(alloc, mybir.MemoryLocationSet):
            continue
        name = alloc.memorylocations[0].name
        if alloc.kind == "ExternalInput":
            if name != partition_name:
                in_names.append(name)
        elif alloc.kind == "ExternalOutput":
            out_names.append(name)
            shape = tuple(alloc.tensor_shape)
            dtype = mybir.dt.np(alloc.dtype)
            out_avals.append(jax.core.ShapedArray(shape, dtype))
            zero_outs.append(np.zeros(shape, dtype))
    n_params = len(in_names)
    all_names = in_names + out_names
    if partition_name is not None:
        all_names = all_names + [partition_name]

    def _body(*args):
        operands = list(args)
        if partition_name is not None:
            operands.append(bass2jax.partition_id_tensor())
        outs = bass2jax._bass_exec_p.bind(
            *operands,
            out_avals=tuple(out_avals),
            in_names=tuple(all_names),
            out_names=tuple(out_names),
            lowering_input_output_aliases=(),
            sim_require_finite=True,
            sim_require_nnan=True,
            nc=nc,
        )
        return tuple(outs)

    devices = jax.devices()[:B]
    mesh = Mesh(np.asarray(devices), ("core",))
    n_outs = len(out_names)
    sharded = jax.jit(
        shard_map(
            _body,
            mesh=mesh,
            in_specs=(PartitionSpec("core"),) * (n_params + n_outs),
            out_specs=(PartitionSpec("core"),) * n_outs,
            check_rep=False,
        ),
        donate_argnums=tuple(range(n_params, n_params + n_outs)),
        keep_unused=True,
    )

    def run(in_maps):
        concat_in = [
            np.concatenate([np.asarray(in_maps[c][nm]) for c in range(B)], axis=0)
            for nm in in_names
        ]
        concat_zeros = [
            np.zeros((B * z.shape[0], *z.shape[1:]), z.dtype) for z in zero_outs
        ]
        out_arrs = sharded(*concat_in, *concat_zeros)
        return [
            {
                nm: np.asarray(out_arrs[i]).reshape(B, *out_avals[i].shape)[c]
                for i, nm in enumerate(out_names)
            }
            for c in range(B)
        ]

    _CACHE[key] = run
    return run


def _run(inputs, trace=False):
    run = _get_runner()
    in_maps = _host_prep(inputs)
    results = run(in_maps)
    # gather: out[b] = sum of the two head-half partials for b's batch pair
    outs = np.zeros((B, S, D), np.float32)
    for c in range(B):
        bp, hh = c % 4, c // 4
        part = np.asarray(results[c]["out"]).reshape(NB, S, D)
        for lb in range(NB):
            outs[2 * bp + lb] += part[lb]
    return outs, None


def kernel(**inputs) -> np.ndarray:
    out, _ = _run(inputs)
    return out
